# revision 18
# baseline (speedup 1.0000x reference)
"""GNN message passing + 3x conv3x3 + leaky-relu, 8 trn2 NeuronCores.

Strategy v4 (node-sharded, 128 nodes/core, (64,64) PE pair tiling):
- PE is LDWEIGHTS-bound for small-channel convs (~0.83ns/weight-col,
  ~34ns floor per LDW). The (64,64) 4-tile config with 2-node block-diag
  pairs is the measured optimum (27.5ns/node-tap-half, exact numerics);
  16-way (32,32) tiling corrupts PSUM accumulations on this HW.
- 8-node groups as (Rp, Cp, j): input partitions 64Rp+32j+ch, output
  partitions 64Cp+32j+ch; conv2/conv3 swap Rp<->Cp roles so no
  cross-partition moves are ever needed.
- conv1 = self pass ((64,64), K rows = both group-parities, other parity
  zero-weighted) + pos/neg pass; conv3 zero-pads out-channels 16..31.
- PSUM: half-image tiles [128, 1024] (2 banks), free = 512*row_pair so
  concurrent row tiles never share a bank; pool bufs=4 pipelines 2
  layers; group layers software-pipelined conv1(g)/conv2(g-1)/conv3(g-2).
- Pooling: host-packed bf16 slab tables (2D degree-trimmed), plain HWDGE
  DMAs, DVE accumulate A += slab; GPSIMD builds padded grids; ACT
  evacuates conv1/conv3 (Prelu), DVE conv2 (mult+max).
"""

import numpy as np

N, C, H, W = 1024, 16, 32, 32
NCORES = 8
NPC = N // NCORES            # nodes per core
NG = NPC // 8                # 8-node groups per core (16)
HP = WP = H + 2
GRID = HP * WP               # 1156
HW = H * W
QH = 16                      # half-image rows
ACC_BF16 = True              # pooled accumulator dtype

_prog_cache = {}


def _make_tile_context(nc):
    """TileContext whose lowering splits multi-sem waits onto nop carriers
    (this walrus build accepts at most one sync wait per instruction) and
    whose tail drain does the same."""
    import concourse.mybir as mybir
    import concourse.tile as tile

    class _TC(tile.TileContext):
        def _lower_ordered_insts(self, ordered):
            for bb_name, insts in ordered.items():
                out = []
                for inst in insts:
                    si = inst.sync_info
                    waits = list(si.on_wait) if si is not None and si.on_wait else []
                    if len(waits) > 1:
                        for w in waits[:-1]:
                            car = mybir.InstNoOp(
                                name=self.nc.get_next_instruction_name(),
                                ins=[], outs=[])
                            car.engine = inst.engine
                            car.sync_info = mybir.SyncInfo(on_wait=[w], on_update=[])
                            self.nc.register_instruction(car, overwrite=True)
                            out.append(car)
                        inst.sync_info = mybir.SyncInfo(
                            on_wait=[waits[-1]],
                            on_update=list(si.on_update) if si.on_update else [])
                    out.append(inst)
                insts[:] = out
            return super()._lower_ordered_insts(ordered)

        def _drain_and_barrier(self, tick_clock, wait_clock):
            clock = tick_clock.global_clock
            allocated = wait_clock.sems.allocated()
            for proc, tick in enumerate(clock):
                if tick > 0 and proc in allocated:
                    n = self.nc.sync.nop(nofuse=True, hint="tailwait")
                    n.wait_op(allocated[proc], tick, "sem-ge")
            self.nc.sync.drain()
            self.nc.all_engine_barrier()
            assert self.sems is not None
            popped = self.nc._tile_sem_poison_stack.pop()
            assert popped is self._sem_poison
            self.nc.clear_and_free_semaphores(list(self.sems.allocated().values()))
            self.nc.all_engine_barrier()

    return _TC(nc)


def _build_program(geom):
    """geom: tuple over groups g of tuple over rounds r of na (active
    slot count 1..8, forced 8 for r<2). Slot s = 2*(2Rp+j) + Cp, degree
    descending. Slab (g,r): full windows piece [32*(na//2), 2048] +
    (na odd) partial piece [32, 1024] at partition base 32*(na//2)."""
    import concourse.bass as bass
    import concourse.mybir as mybir

    f32 = mybir.dt.float32
    bf16 = mybir.dt.bfloat16
    adt = bf16 if ACC_BF16 else f32
    AF = mybir.ActivationFunctionType
    ALU = mybir.AluOpType

    nc = bass.Bass()
    g_d = {}
    for g in range(NG):
        for r, na in enumerate(geom[g]):
            fw, odd = na // 2, na % 2
            if fw > 0:
                g_d[(g, r, 'a')] = nc.dram_tensor(
                    f"g{g}_{r}a", [32 * fw, 2048], bf16,
                    kind="ExternalInput")
            if odd:
                g_d[(g, r, 'b')] = nc.dram_tensor(
                    f"g{g}_{r}b", [32, 1024], bf16, kind="ExternalInput")
    fown_d = nc.dram_tensor("fown", [128, (NG // 2) * 2048], bf16,
                            kind="ExternalInput")
    w1p_d = nc.dram_tensor("w1p", [128, 9 * 64], bf16, kind="ExternalInput")
    w1s_d = nc.dram_tensor("w1s", [128, 2 * 9 * 64], bf16,
                           kind="ExternalInput")
    w2_d = nc.dram_tensor("w2l", [128, 9 * 64], bf16, kind="ExternalInput")
    w3_d = nc.dram_tensor("w3l", [128, 9 * 64], bf16, kind="ExternalInput")
    y_d = nc.dram_tensor("y", [128, NG * 2048], bf16, kind="ExternalOutput")

    tc = _make_tile_context(nc)
    with tc:
        with (tc.tile_pool(name="cw", bufs=1) as cw,
              tc.tile_pool(name="stp", bufs=12) as stp,
              tc.tile_pool(name="apool", bufs=3) as apool,
              tc.tile_pool(name="sgp", bufs=2) as sgp,
              tc.tile_pool(name="png", bufs=3) as png,
              tc.tile_pool(name="x2p", bufs=3) as x2p,
              tc.tile_pool(name="x3p", bufs=3) as x3p,
              tc.tile_pool(name="osbp", bufs=3) as osbp,
              tc.tile_pool(name="psp", bufs=4, space="PSUM") as psp):
            w1p_t = cw.tile([128, 9 * 64], bf16)
            nc.scalar.dma_start(out=w1p_t[:], in_=w1p_d[:])
            w1s_t = cw.tile([128, 2 * 9 * 64], bf16)
            nc.scalar.dma_start(out=w1s_t[:], in_=w1s_d[:])
            w2_t = cw.tile([128, 9 * 64], bf16)
            nc.scalar.dma_start(out=w2_t[:], in_=w2_d[:])
            w3_t = cw.tile([128, 9 * 64], bf16)
            nc.scalar.dma_start(out=w3_t[:], in_=w3_d[:])
            fo_t = cw.tile([128, (NG // 2) * 2048], bf16)
            half = (NG // 2) * 1024
            nc.scalar.dma_start(out=fo_t[:, 0:2048], in_=fown_d[:, 0:2048])
            nc.scalar.dma_start(out=fo_t[:, 2048:half],
                                in_=fown_d[:, 2048:half])
            nc.scalar.dma_start(out=fo_t[:, half:2 * half],
                                in_=fown_d[:, half:2 * half])

            # gather slab DMAs (sync queue), consumed in order by DVE
            stages = {}
            for g in range(NG):
                for r, na in enumerate(geom[g]):
                    fw, odd = na // 2, na % 2
                    st = stp.tile([128, 2048], bf16, tag="st",
                                  name=f"st{g}_{r}")
                    if fw > 0:
                        nc.sync.dma_start(out=st[0:32 * fw, :],
                                          in_=g_d[(g, r, 'a')][:])
                    if odd:
                        nc.sync.dma_start(
                            out=st[32 * fw:32 * fw + 32, 0:1024],
                            in_=g_d[(g, r, 'b')][:])
                    stages[(g, r)] = st

            memset_count = {}

            def fresh_grid(pool, nm, width, bufs):
                t = pool.tile([128, width], bf16, tag=nm, name=f"{nm}_t")
                ccount = memset_count.get(nm, 0)
                if ccount < bufs:
                    nc.vector.memset(t[:], 0.0)
                    memset_count[nm] = ccount + 1
                return t

            # self grids per group-pair, built once when first needed
            sg_tiles = {}

            def get_sg(pair):
                if pair in sg_tiles:
                    return sg_tiles[pair]
                sg = fresh_grid(sgp, "sg", 2 * GRID, 2)
                sg3 = sg[:].rearrange("p (cp h w) -> p cp h w", cp=2, h=HP)
                fo3 = fo_t[:, 2048 * pair:2048 * (pair + 1)].rearrange(
                    "p (cp h w) -> p cp h w", cp=2, h=H)
                for cp in range(2):
                    nc.scalar.activation(
                        out=sg3[:, cp, 1:H + 1, 1:W + 1],
                        in_=fo3[:, cp, :, :], func=AF.Copy)
                sg_tiles[pair] = sg3
                if pair - 1 in sg_tiles:
                    del sg_tiles[pair - 1]  # allow pool recycle tracking
                return sg3

            def add_split(out_full, in0_full, in1_full, fw, odd):
                # windows 0..fw-1 full-width + odd half-window; DVE takes
                # all but the last full window, GPSIMD the last (it runs
                # ~2.8x slower, so ~25% of the volume)
                gw = 1 if fw >= 3 else 0
                dw = fw - gw
                if dw > 0:
                    nc.vector.tensor_add(
                        out=out_full[0:32 * dw, :],
                        in0=in0_full[0:32 * dw, :],
                        in1=in1_full[0:32 * dw, :])
                if gw:
                    nc.gpsimd.tensor_add(
                        out=out_full[32 * dw:32 * fw, :],
                        in0=in0_full[32 * dw:32 * fw, :],
                        in1=in1_full[32 * dw:32 * fw, :])
                if odd:
                    nc.vector.tensor_add(
                        out=out_full[32 * fw:32 * fw + 32, 0:1024],
                        in0=in0_full[32 * fw:32 * fw + 32, 0:1024],
                        in1=in1_full[32 * fw:32 * fw + 32, 0:1024])

            def pool_gather(g):
                # accumulate rounds 0..rmax-2 into A; round rmax-1 is
                # folded into the grid-interior write by conv1
                gg = geom[g]
                if len(gg) == 2:
                    return stages[(g, 0)], stages[(g, 1)], 8
                A = apool.tile([128, 2048], adt, tag="A", name=f"A{g}")
                s0, s1 = stages[(g, 0)], stages[(g, 1)]
                add_split(A[:], s0[:], s1[:], 4, 0)
                for r in range(2, len(gg) - 1):
                    na = gg[r]
                    st = stages[(g, r)]
                    add_split(A[:], st[:], A[:], na // 2, na % 2)
                return A, stages[(g, len(gg) - 1)], gg[-1]

            x2g = {}
            x3g = {}
            osbg = {}

            # PE warm-up: keep HAM hot through the pipeline-fill phase.
            # Garbage matmuls on the first gather slab into a scratch
            # PSUM tile that is never read.
            wpt = psp.tile([128, 1024], f32, tag="pt", name="warmpt")
            s0w = stages[(0, 0)]
            for i in range(60):
                nc.tensor.matmul(
                    out=wpt[0:64, 0:512],
                    lhsT=w2_t[0:64, 0:64],
                    rhs=s0w[0:64, 512 * (i % 3):512 * (i % 3) + 512],
                    start=True, stop=True,
                    tile_position=(0, 0), skip_group_check=True)

            def conv1(g):
                A, last, na_last = pool_gather(g)
                sg3 = get_sg(g // 2)
                pn = fresh_grid(png, "pn", 2 * GRID, 3)
                pn3 = pn[:].rearrange("p (cp h w) -> p cp h w", cp=2, h=HP)
                A3 = A[:].rearrange("p (cp h w) -> p cp h w", cp=2, h=H)
                L3 = last[:].rearrange("p (cp h w) -> p cp h w", cp=2, h=H)
                fw, odd = na_last // 2, na_last % 2
                for cp in range(2):
                    # prefix windows covered by the last round: A + last
                    nw_cp = fw + (odd if cp == 0 else 0)
                    gw = 1 if nw_cp >= 3 else 0
                    dw = nw_cp - gw
                    if dw > 0:
                        nc.vector.tensor_add(
                            out=pn3[0:32 * dw, cp, 1:H + 1, 1:W + 1],
                            in0=A3[0:32 * dw, cp, :, :],
                            in1=L3[0:32 * dw, cp, :, :])
                    if gw:
                        nc.gpsimd.tensor_add(
                            out=pn3[32 * dw:32 * nw_cp, cp,
                                    1:H + 1, 1:W + 1],
                            in0=A3[32 * dw:32 * nw_cp, cp, :, :],
                            in1=L3[32 * dw:32 * nw_cp, cp, :, :])
                    for wdw in range(nw_cp, 4):
                        nc.vector.tensor_copy(
                            out=pn3[32 * wdw:32 * wdw + 32, cp,
                                    1:H + 1, 1:W + 1],
                            in_=A3[32 * wdw:32 * wdw + 32, cp, :, :])
                x2 = fresh_grid(x2p, "x2", 2 * GRID, 3)
                x23 = x2[:].rearrange("p (rp h w) -> p rp h w", rp=2, h=HP)
                par = g % 2
                for h in range(2):
                    pt = psp.tile([128, 1024], f32, tag="pt",
                                  name=f"pt1_{g}_{h}")
                    # self pass: K=64 rows (both parities; other parity
                    # zero-weighted), M=64
                    for t in range(9):
                        dy, dx = t // 3, t % 3
                        for Rp in range(2):
                            for Cp in range(2):
                                nc.tensor.matmul(
                                    out=pt[64 * Cp:64 * Cp + 64,
                                           512 * Rp:512 * Rp + 512],
                                    lhsT=w1s_t[64 * Rp:64 * Rp + 64,
                                               576 * par + 64 * t:
                                               576 * par + 64 * t + 64],
                                    rhs=sg3[64 * Rp:64 * Rp + 64, Cp,
                                            QH * h + dy:QH * h + dy + QH,
                                            dx:dx + W],
                                    start=(t == 0), stop=False,
                                    tile_position=(64 * Rp, 64 * Cp),
                                    skip_group_check=True)
                    # pos/neg pass
                    for t in range(9):
                        dy, dx = t // 3, t % 3
                        for Rp in range(2):
                            for Cp in range(2):
                                nc.tensor.matmul(
                                    out=pt[64 * Cp:64 * Cp + 64,
                                           512 * Rp:512 * Rp + 512],
                                    lhsT=w1p_t[64 * Rp:64 * Rp + 64,
                                               64 * t:64 * t + 64],
                                    rhs=pn3[64 * Rp:64 * Rp + 64, Cp,
                                            QH * h + dy:QH * h + dy + QH,
                                            dx:dx + W],
                                    start=False, stop=(t == 8),
                                    tile_position=(64 * Rp, 64 * Cp),
                                    skip_group_check=True)
                    ptr = pt[:].rearrange("p (rp h w) -> p rp h w",
                                          rp=2, h=QH)
                    for rp in range(2):
                        nc.scalar.activation(
                            out=x23[:, rp, 1 + QH * h:1 + QH * h + QH,
                                    1:W + 1],
                            in_=ptr[:, rp, :, :], func=AF.Prelu, alpha=0.1)
                x2g[g] = x23

            def conv2(g):
                x23 = x2g.pop(g)
                x3 = fresh_grid(x3p, "x3", 2 * GRID, 3)
                x33 = x3[:].rearrange("p (cp h w) -> p cp h w", cp=2, h=HP)
                for h in range(2):
                    pt = psp.tile([128, 1024], f32, tag="pt",
                                  name=f"pt2_{g}_{h}")
                    for t in range(9):
                        dy, dx = t // 3, t % 3
                        for Cp in range(2):
                            for Rp in range(2):
                                nc.tensor.matmul(
                                    out=pt[64 * Rp:64 * Rp + 64,
                                           512 * Cp:512 * Cp + 512],
                                    lhsT=w2_t[64 * Cp:64 * Cp + 64,
                                              64 * t:64 * t + 64],
                                    rhs=x23[64 * Cp:64 * Cp + 64, Rp,
                                            QH * h + dy:QH * h + dy + QH,
                                            dx:dx + W],
                                    start=(t == 0), stop=(t == 8),
                                    tile_position=(64 * Cp, 64 * Rp),
                                    skip_group_check=True)
                    ptr = pt[:].rearrange("p (cp h w) -> p cp h w",
                                          cp=2, h=QH)
                    for cp in range(2):
                        x3v = x33[:, cp, 1 + QH * h:1 + QH * h + QH,
                                  1:W + 1]
                        pv = ptr[:, cp, :, :]
                        nc.vector.tensor_scalar(
                            out=x3v, in0=pv, scalar1=0.1, scalar2=None,
                            op0=ALU.mult)
                        nc.vector.scalar_tensor_tensor(
                            out=x3v, in0=x3v, scalar=0.0, in1=pv,
                            op0=ALU.bypass, op1=ALU.max)
                x3g[g] = x33

            def conv3(g):
                x33 = x3g.pop(g)
                osb = osbp.tile([128, 2048], bf16, tag="osb",
                                name=f"osb{g}")
                osb4 = osb[:].rearrange("p (rp h w) -> p rp h w",
                                        rp=2, h=H)
                for h in range(2):
                    pt = psp.tile([128, 1024], f32, tag="pt",
                                  name=f"pt3_{g}_{h}")
                    for t in range(9):
                        dy, dx = t // 3, t % 3
                        for Rp in range(2):
                            for Cp in range(2):
                                nc.tensor.matmul(
                                    out=pt[64 * Cp:64 * Cp + 64,
                                           512 * Rp:512 * Rp + 512],
                                    lhsT=w3_t[64 * Rp:64 * Rp + 64,
                                              64 * t:64 * t + 64],
                                    rhs=x33[64 * Rp:64 * Rp + 64, Cp,
                                            QH * h + dy:QH * h + dy + QH,
                                            dx:dx + W],
                                    start=(t == 0), stop=(t == 8),
                                    tile_position=(64 * Rp, 64 * Cp),
                                    skip_group_check=True)
                    ptr = pt[:].rearrange("p (rp h w) -> p rp h w",
                                          rp=2, h=QH)
                    for rp in range(2):
                        nc.scalar.activation(
                            out=osb4[:, rp, QH * h:QH * h + QH, :],
                            in_=ptr[:, rp, :, :], func=AF.Prelu, alpha=0.1)
                nc.scalar.dma_start(out=y_d[:, 2048 * g:2048 * (g + 1)],
                                    in_=osb[:])

            # software-pipelined group loop
            for g in range(NG + 2):
                if g < NG:
                    conv1(g)
                if 1 <= g < NG + 1:
                    conv2(g - 1)
                if g >= 2:
                    conv3(g - 2)
    return nc


def _host_prep(feats, edges, w1, b1, w2, b2, w3, b3):
    import ml_dtypes

    feats = np.ascontiguousarray(np.asarray(feats, dtype=np.float32))
    edges = np.asarray(edges)
    w1 = np.asarray(w1, dtype=np.float32)
    w2 = np.asarray(w2, dtype=np.float32)
    w3 = np.asarray(w3, dtype=np.float32)

    contrib = [([], []) for _ in range(N)]
    for s, sg, d in edges.tolist():
        si = 0 if sg > 0 else 1
        contrib[d][si].append(s)
        contrib[s][si].append(d)
    md = np.array([max(len(contrib[n][0]), len(contrib[n][1]), 1)
                   for n in range(N)])

    # groups ascending by max-degree; slots within a group descending;
    # slot s = 2*(2Rp+j) + Cp
    slots = []
    for k in range(NCORES):
        nodes = sorted(range(NPC * k, NPC * (k + 1)), key=lambda n: md[n])
        gs = []
        for g in range(NG):
            gn = sorted(nodes[8 * g:8 * g + 8], key=lambda n: -md[n])
            gs.append(gn)
        slots.append(gs)

    geom = []
    for g in range(NG):
        rmax = max(max(md[slots[k][g][s]] for s in range(8))
                   for k in range(NCORES))
        rmax = max(int(rmax), 2)
        nas = []
        for r in range(rmax):
            if r < 2:
                nas.append(8)
            else:
                na = max(sum(1 for s in range(8) if md[slots[k][g][s]] > r)
                         for k in range(NCORES))
                nas.append(max(na, 1))
        geom.append(tuple(nas))
    geom = tuple(geom)

    featsN = feats.reshape(N, C, HW)
    tabN = np.concatenate([featsN, np.zeros((1, C, HW), np.float32)],
                          axis=0)
    tab_bf = tabN.astype(ml_dtypes.bfloat16)

    # weights (lhsT layouts, [64,64] pair tiles, replicated over row pairs)
    w1p = np.zeros((64, 9 * 64), np.float32)
    w1s = np.zeros((64, 2 * 9 * 64), np.float32)   # two parity variants
    w2l = np.zeros((64, 9 * 64), np.float32)
    w3l = np.zeros((64, 9 * 64), np.float32)
    for t in range(9):
        dy, dx = t // 3, t % 3
        for j in range(2):
            # pn: rows 32j+16sg+c -> cols 32j+m
            w1p[32 * j:32 * j + 32, 64 * t + 32 * j:64 * t + 32 * j + 32] = \
                w1[:, C:3 * C, dy, dx].T
            # w2: rows 32j+k -> cols 32j+m
            w2l[32 * j:32 * j + 32, 64 * t + 32 * j:64 * t + 32 * j + 32] = \
                w2[:, :, dy, dx].T
            # w3: rows 32j+k -> cols 32j+c (c<16)
            w3l[32 * j:32 * j + 32, 64 * t + 32 * j:64 * t + 32 * j + 16] = \
                w3[:, :, dy, dx].T
            # self variants: par p: rows 32p+16j+c -> cols 32j+m
            for p in range(2):
                w1s[32 * p + 16 * j:32 * p + 16 * j + 16,
                    576 * p + 64 * t + 32 * j:
                    576 * p + 64 * t + 32 * j + 32] = w1[:, 0:C, dy, dx].T
    rep = lambda a: np.ascontiguousarray(
        np.tile(a, (2, 1)).astype(ml_dtypes.bfloat16))

    in_maps = []
    for k in range(NCORES):
        m = {"w1p": rep(w1p), "w1s": rep(w1s), "w2l": rep(w2l),
             "w3l": rep(w3l)}
        # fown: pair P = g//2: p = 64Rp + 32*(g%2) + 16j + c,
        # free = 2048P + 1024Cp + px
        fo = np.zeros((128, (NG // 2) * 2048), ml_dtypes.bfloat16)
        for g in range(NG):
            for s in range(8):
                w_ = s // 2
                Rp, j, Cp = w_ // 2, w_ % 2, s % 2
                nsrc = slots[k][g][s]
                p0 = 64 * Rp + 32 * (g % 2) + 16 * j
                fo[p0:p0 + 16,
                   2048 * (g // 2) + 1024 * Cp:
                   2048 * (g // 2) + 1024 * Cp + 1024] = tab_bf[nsrc]
        m["fown"] = np.ascontiguousarray(fo)
        # gather slabs: row p = 32*(2Rp+j) + 16sg + c, col = 1024*Cp + px
        for g in range(NG):
            for r, na in enumerate(geom[g]):
                fw, odd = na // 2, na % 2
                nw = fw + odd
                srcs = np.full((2 * nw, 2), N, np.int64)  # [(w,sg), Cp]
                for s in range(8 if r < 2 else na):
                    w_, Cp = s // 2, s % 2
                    if w_ >= nw:
                        continue
                    nsrc = slots[k][g][s]
                    for sg in range(2):
                        lst = contrib[nsrc][sg]
                        if r < len(lst):
                            srcs[2 * w_ + sg, Cp] = lst[r]
                arr = tab_bf[srcs]  # [2nw, 2, C, HW]
                full = np.ascontiguousarray(
                    arr.transpose(0, 2, 1, 3).reshape(32 * nw, 2048))
                if fw > 0:
                    m[f"g{g}_{r}a"] = np.ascontiguousarray(
                        full[0:32 * fw, :])
                if odd:
                    m[f"g{g}_{r}b"] = np.ascontiguousarray(
                        full[32 * fw:32 * fw + 32, 0:1024])
        in_maps.append(m)
    return in_maps, slots, geom


def kernel(feats, edges, w1, b1, w2, b2, w3, b3):
    from concourse.bass_utils import run_bass_kernel_spmd

    with_bias = bool(np.any(np.asarray(b1)) or np.any(np.asarray(b2))
                     or np.any(np.asarray(b3)))
    assert not with_bias, "nonzero conv biases not implemented"

    in_maps, slots, geom = _host_prep(feats, edges, w1, b1, w2, b2, w3, b3)

    nc = _prog_cache.get(geom)
    if nc is None:
        nc = _build_program(geom)
        _prog_cache[geom] = nc

    import os
    trace = bool(os.environ.get("KERNEL_TRACE"))
    res = run_bass_kernel_spmd(nc, in_maps, core_ids=list(range(NCORES)),
                               trace=trace)
    if trace:
        global last_results
        last_results = res

    out = np.empty((N, C, H, W), np.float32)
    for k in range(NCORES):
        yk = np.asarray(res.results[k]["y"]).astype(np.float32)
        for g in range(NG):
            for s in range(8):
                w_ = s // 2
                Rp, j, Cp = w_ // 2, w_ % 2, s % 2
                n = slots[k][g][s]
                out[n] = yk[64 * Cp + 32 * j:64 * Cp + 32 * j + C,
                            2048 * g + 1024 * Rp:
                            2048 * g + 1024 * Rp + 1024].reshape(C, H, W)
    return out


# revision 19
# speedup vs baseline: 1.5395x; 1.5395x over previous
"""GNN message passing + 3x conv3x3 + leaky-relu, 8 trn2 NeuronCores.

Strategy v4 (node-sharded, 128 nodes/core, (64,64) PE pair tiling):
- PE is LDWEIGHTS-bound for small-channel convs (~0.83ns/weight-col,
  ~34ns floor per LDW). The (64,64) 4-tile config with 2-node block-diag
  pairs is the measured optimum (27.5ns/node-tap-half, exact numerics);
  16-way (32,32) tiling corrupts PSUM accumulations on this HW.
- 8-node groups as (Rp, Cp, j): input partitions 64Rp+32j+ch, output
  partitions 64Cp+32j+ch; conv2/conv3 swap Rp<->Cp roles so no
  cross-partition moves are ever needed.
- conv1 = self pass ((64,64), K rows = both group-parities, other parity
  zero-weighted) + pos/neg pass; conv3 zero-pads out-channels 16..31.
- PSUM: half-image tiles [128, 1024] (2 banks), free = 512*row_pair so
  concurrent row tiles never share a bank; pool bufs=4 pipelines 2
  layers; group layers software-pipelined conv1(g)/conv2(g-1)/conv3(g-2).
- Pooling: host-packed bf16 slab tables (2D degree-trimmed), plain HWDGE
  DMAs, DVE accumulate A += slab; GPSIMD builds padded grids; ACT
  evacuates conv1/conv3 (Prelu), DVE conv2 (mult+max).
"""

import numpy as np

N, C, H, W = 1024, 16, 32, 32
NCORES = 8
NPC = N // NCORES            # nodes per core
NG = NPC // 8                # 8-node groups per core (16)
HP = WP = H + 2
GRID = HP * WP               # 1156
HW = H * W
QH = 16                      # half-image rows
ACC_BF16 = True              # pooled accumulator dtype

_prog_cache = {}


def _make_tile_context(nc):
    """TileContext whose lowering splits multi-sem waits onto nop carriers
    (this walrus build accepts at most one sync wait per instruction) and
    whose tail drain does the same."""
    import concourse.mybir as mybir
    import concourse.tile as tile

    class _TC(tile.TileContext):
        def _lower_ordered_insts(self, ordered):
            for bb_name, insts in ordered.items():
                out = []
                for inst in insts:
                    si = inst.sync_info
                    waits = list(si.on_wait) if si is not None and si.on_wait else []
                    if len(waits) > 1:
                        for w in waits[:-1]:
                            car = mybir.InstNoOp(
                                name=self.nc.get_next_instruction_name(),
                                ins=[], outs=[])
                            car.engine = inst.engine
                            car.sync_info = mybir.SyncInfo(on_wait=[w], on_update=[])
                            self.nc.register_instruction(car, overwrite=True)
                            out.append(car)
                        inst.sync_info = mybir.SyncInfo(
                            on_wait=[waits[-1]],
                            on_update=list(si.on_update) if si.on_update else [])
                    out.append(inst)
                insts[:] = out
            return super()._lower_ordered_insts(ordered)

        def _drain_and_barrier(self, tick_clock, wait_clock):
            clock = tick_clock.global_clock
            allocated = wait_clock.sems.allocated()
            for proc, tick in enumerate(clock):
                if tick > 0 and proc in allocated:
                    n = self.nc.sync.nop(nofuse=True, hint="tailwait")
                    n.wait_op(allocated[proc], tick, "sem-ge")
            self.nc.sync.drain()
            self.nc.all_engine_barrier()
            assert self.sems is not None
            popped = self.nc._tile_sem_poison_stack.pop()
            assert popped is self._sem_poison
            self.nc.clear_and_free_semaphores(list(self.sems.allocated().values()))
            self.nc.all_engine_barrier()

    return _TC(nc)


def _build_program(geom):
    """geom: tuple over groups g of tuple over rounds r of na (active
    slot count 1..8, forced 8 for r<2). Slot s = 2*(2Rp+j) + Cp, degree
    descending. Slab (g,r): full windows piece [32*(na//2), 2048] +
    (na odd) partial piece [32, 1024] at partition base 32*(na//2)."""
    import concourse.bass as bass
    import concourse.mybir as mybir

    f32 = mybir.dt.float32
    bf16 = mybir.dt.bfloat16
    adt = bf16 if ACC_BF16 else f32
    AF = mybir.ActivationFunctionType
    ALU = mybir.AluOpType

    nc = bass.Bass()
    g_d = {}
    for g in range(NG):
        for r, na in enumerate(geom[g]):
            fw, odd = na // 2, na % 2
            if fw > 0:
                g_d[(g, r, 'a')] = nc.dram_tensor(
                    f"g{g}_{r}a", [32 * fw, 2048], bf16,
                    kind="ExternalInput")
            if odd:
                g_d[(g, r, 'b')] = nc.dram_tensor(
                    f"g{g}_{r}b", [32, 1024], bf16, kind="ExternalInput")
    fown_d = nc.dram_tensor("fown", [128, (NG // 2) * 2048], bf16,
                            kind="ExternalInput")
    w1p_d = nc.dram_tensor("w1p", [128, 9 * 64], bf16, kind="ExternalInput")
    w1s_d = nc.dram_tensor("w1s", [128, 2 * 9 * 64], bf16,
                           kind="ExternalInput")
    w2_d = nc.dram_tensor("w2l", [128, 9 * 64], bf16, kind="ExternalInput")
    w3_d = nc.dram_tensor("w3l", [128, 9 * 64], bf16, kind="ExternalInput")
    y_d = nc.dram_tensor("y", [128, NG * 2048], bf16, kind="ExternalOutput")

    tc = _make_tile_context(nc)
    with tc:
        with (tc.tile_pool(name="cw", bufs=1) as cw,
              tc.tile_pool(name="stp", bufs=12) as stp,
              tc.tile_pool(name="apool", bufs=4) as apool,
              tc.tile_pool(name="sgp", bufs=2) as sgp,
              tc.tile_pool(name="png", bufs=3) as png,
              tc.tile_pool(name="x2p", bufs=3) as x2p,
              tc.tile_pool(name="x3p", bufs=3) as x3p,
              tc.tile_pool(name="osbp", bufs=3) as osbp,
              tc.tile_pool(name="psp", bufs=4, space="PSUM") as psp):
            w1p_t = cw.tile([128, 9 * 64], bf16)
            nc.scalar.dma_start(out=w1p_t[:], in_=w1p_d[:])
            w1s_t = cw.tile([128, 2 * 9 * 64], bf16)
            nc.scalar.dma_start(out=w1s_t[:], in_=w1s_d[:])
            w2_t = cw.tile([128, 9 * 64], bf16)
            nc.scalar.dma_start(out=w2_t[:], in_=w2_d[:])
            w3_t = cw.tile([128, 9 * 64], bf16)
            nc.scalar.dma_start(out=w3_t[:], in_=w3_d[:])
            fo_t = cw.tile([128, (NG // 2) * 2048], bf16)
            half = (NG // 2) * 1024
            nc.scalar.dma_start(out=fo_t[:, 0:2048], in_=fown_d[:, 0:2048])
            nc.scalar.dma_start(out=fo_t[:, 2048:half],
                                in_=fown_d[:, 2048:half])
            nc.scalar.dma_start(out=fo_t[:, half:2 * half],
                                in_=fown_d[:, half:2 * half])

            # gather slab DMAs (sync queue), consumed in order by DVE
            stages = {}
            for g in range(NG):
                for r, na in enumerate(geom[g]):
                    fw, odd = na // 2, na % 2
                    st = stp.tile([128, 2048], bf16, tag="st",
                                  name=f"st{g}_{r}")
                    if fw > 0:
                        nc.sync.dma_start(out=st[0:32 * fw, :],
                                          in_=g_d[(g, r, 'a')][:])
                    if odd:
                        nc.sync.dma_start(
                            out=st[32 * fw:32 * fw + 32, 0:1024],
                            in_=g_d[(g, r, 'b')][:])
                    stages[(g, r)] = st

            memset_count = {}

            def fresh_grid(pool, nm, width, bufs):
                t = pool.tile([128, width], bf16, tag=nm, name=f"{nm}_t")
                ccount = memset_count.get(nm, 0)
                if ccount < bufs:
                    nc.vector.memset(t[:], 0.0)
                    memset_count[nm] = ccount + 1
                return t

            # self grids per group-pair, built once when first needed
            sg_tiles = {}

            def get_sg(pair):
                if pair in sg_tiles:
                    return sg_tiles[pair]
                sg = fresh_grid(sgp, "sg", 2 * GRID, 2)
                sg3 = sg[:].rearrange("p (cp h w) -> p cp h w", cp=2, h=HP)
                fo3 = fo_t[:, 2048 * pair:2048 * (pair + 1)].rearrange(
                    "p (cp h w) -> p cp h w", cp=2, h=H)
                for cp in range(2):
                    nc.scalar.activation(
                        out=sg3[:, cp, 1:H + 1, 1:W + 1],
                        in_=fo3[:, cp, :, :], func=AF.Copy)
                sg_tiles[pair] = sg3
                if pair - 1 in sg_tiles:
                    del sg_tiles[pair - 1]  # allow pool recycle tracking
                return sg3

            def add_split(out_full, in0_full, in1_full, fw, odd):
                # windows 0..fw-1 full-width + odd half-window; DVE takes
                # all but the last full window, GPSIMD the last (it runs
                # ~2.8x slower, so ~25% of the volume)
                if fw > 0:
                    nc.vector.tensor_add(
                        out=out_full[0:32 * fw, :],
                        in0=in0_full[0:32 * fw, :],
                        in1=in1_full[0:32 * fw, :])
                if odd:
                    nc.vector.tensor_add(
                        out=out_full[32 * fw:32 * fw + 32, 0:1024],
                        in0=in0_full[32 * fw:32 * fw + 32, 0:1024],
                        in1=in1_full[32 * fw:32 * fw + 32, 0:1024])

            def pool_gather(g):
                # accumulate rounds 0..rmax-2 into A; round rmax-1 is
                # folded into the grid-interior write by conv1
                gg = geom[g]
                if len(gg) == 2:
                    return stages[(g, 0)], stages[(g, 1)], 8
                A = apool.tile([128, 2048], adt, tag="A", name=f"A{g}")
                s0, s1 = stages[(g, 0)], stages[(g, 1)]
                add_split(A[:], s0[:], s1[:], 4, 0)
                for r in range(2, len(gg) - 1):
                    na = gg[r]
                    st = stages[(g, r)]
                    add_split(A[:], st[:], A[:], na // 2, na % 2)
                return A, stages[(g, len(gg) - 1)], gg[-1]

            x2g = {}
            x3g = {}
            osbg = {}

            # PE warm-up: keep HAM hot through the pipeline-fill phase.
            # Garbage matmuls on the first gather slab into a scratch
            # PSUM tile that is never read.
            wpt = psp.tile([128, 1024], f32, tag="pt", name="warmpt")
            s0w = stages[(0, 0)]
            for i in range(60):
                nc.tensor.matmul(
                    out=wpt[0:64, 0:512],
                    lhsT=w2_t[0:64, 0:64],
                    rhs=s0w[0:64, 512 * (i % 3):512 * (i % 3) + 512],
                    start=True, stop=True,
                    tile_position=(0, 0), skip_group_check=True)

            def conv1(g, A, last, na_last):
                sg3 = get_sg(g // 2)
                pn = fresh_grid(png, "pn", 2 * GRID, 3)
                pn3 = pn[:].rearrange("p (cp h w) -> p cp h w", cp=2, h=HP)
                A3 = A[:].rearrange("p (cp h w) -> p cp h w", cp=2, h=H)
                L3 = last[:].rearrange("p (cp h w) -> p cp h w", cp=2, h=H)
                fw, odd = na_last // 2, na_last % 2
                for cp in range(2):
                    # prefix windows covered by the last round: A + last
                    nw_cp = fw + (odd if cp == 0 else 0)
                    if nw_cp > 0:
                        nc.vector.tensor_add(
                            out=pn3[0:32 * nw_cp, cp, 1:H + 1, 1:W + 1],
                            in0=A3[0:32 * nw_cp, cp, :, :],
                            in1=L3[0:32 * nw_cp, cp, :, :])
                    for wdw in range(nw_cp, 4):
                        nc.vector.tensor_copy(
                            out=pn3[32 * wdw:32 * wdw + 32, cp,
                                    1:H + 1, 1:W + 1],
                            in_=A3[32 * wdw:32 * wdw + 32, cp, :, :])
                x2 = fresh_grid(x2p, "x2", 2 * GRID, 3)
                x23 = x2[:].rearrange("p (rp h w) -> p rp h w", rp=2, h=HP)
                par = g % 2
                for h in range(2):
                    pt = psp.tile([128, 1024], f32, tag="pt",
                                  name=f"pt1_{g}_{h}")
                    # self pass: K=64 rows (both parities; other parity
                    # zero-weighted), M=64
                    for t in range(9):
                        dy, dx = t // 3, t % 3
                        for Rp in range(2):
                            for Cp in range(2):
                                nc.tensor.matmul(
                                    out=pt[64 * Cp:64 * Cp + 64,
                                           512 * Rp:512 * Rp + 512],
                                    lhsT=w1s_t[64 * Rp:64 * Rp + 64,
                                               576 * par + 64 * t:
                                               576 * par + 64 * t + 64],
                                    rhs=sg3[64 * Rp:64 * Rp + 64, Cp,
                                            QH * h + dy:QH * h + dy + QH,
                                            dx:dx + W],
                                    start=(t == 0), stop=False,
                                    tile_position=(64 * Rp, 64 * Cp),
                                    skip_group_check=True)
                    # pos/neg pass
                    for t in range(9):
                        dy, dx = t // 3, t % 3
                        for Rp in range(2):
                            for Cp in range(2):
                                nc.tensor.matmul(
                                    out=pt[64 * Cp:64 * Cp + 64,
                                           512 * Rp:512 * Rp + 512],
                                    lhsT=w1p_t[64 * Rp:64 * Rp + 64,
                                               64 * t:64 * t + 64],
                                    rhs=pn3[64 * Rp:64 * Rp + 64, Cp,
                                            QH * h + dy:QH * h + dy + QH,
                                            dx:dx + W],
                                    start=False, stop=(t == 8),
                                    tile_position=(64 * Rp, 64 * Cp),
                                    skip_group_check=True)
                    ptr = pt[:].rearrange("p (rp h w) -> p rp h w",
                                          rp=2, h=QH)
                    for rp in range(2):
                        nc.scalar.activation(
                            out=x23[:, rp, 1 + QH * h:1 + QH * h + QH,
                                    1:W + 1],
                            in_=ptr[:, rp, :, :], func=AF.Prelu, alpha=0.1)
                x2g[g] = x23

            def conv2(g):
                x23 = x2g.pop(g)
                x3 = fresh_grid(x3p, "x3", 2 * GRID, 3)
                x33 = x3[:].rearrange("p (cp h w) -> p cp h w", cp=2, h=HP)
                for h in range(2):
                    pt = psp.tile([128, 1024], f32, tag="pt",
                                  name=f"pt2_{g}_{h}")
                    for t in range(9):
                        dy, dx = t // 3, t % 3
                        for Cp in range(2):
                            for Rp in range(2):
                                nc.tensor.matmul(
                                    out=pt[64 * Rp:64 * Rp + 64,
                                           512 * Cp:512 * Cp + 512],
                                    lhsT=w2_t[64 * Cp:64 * Cp + 64,
                                              64 * t:64 * t + 64],
                                    rhs=x23[64 * Cp:64 * Cp + 64, Rp,
                                            QH * h + dy:QH * h + dy + QH,
                                            dx:dx + W],
                                    start=(t == 0), stop=(t == 8),
                                    tile_position=(64 * Cp, 64 * Rp),
                                    skip_group_check=True)
                    ptr = pt[:].rearrange("p (cp h w) -> p cp h w",
                                          cp=2, h=QH)
                    for cp in range(2):
                        x3v = x33[:, cp, 1 + QH * h:1 + QH * h + QH,
                                  1:W + 1]
                        pv = ptr[:, cp, :, :]
                        nc.vector.tensor_scalar(
                            out=x3v, in0=pv, scalar1=0.1, scalar2=None,
                            op0=ALU.mult)
                        nc.vector.scalar_tensor_tensor(
                            out=x3v, in0=x3v, scalar=0.0, in1=pv,
                            op0=ALU.bypass, op1=ALU.max)
                x3g[g] = x33

            def conv3(g):
                x33 = x3g.pop(g)
                osb = osbp.tile([128, 2048], bf16, tag="osb",
                                name=f"osb{g}")
                osb4 = osb[:].rearrange("p (rp h w) -> p rp h w",
                                        rp=2, h=H)
                for h in range(2):
                    pt = psp.tile([128, 1024], f32, tag="pt",
                                  name=f"pt3_{g}_{h}")
                    for t in range(9):
                        dy, dx = t // 3, t % 3
                        for Rp in range(2):
                            for Cp in range(2):
                                nc.tensor.matmul(
                                    out=pt[64 * Cp:64 * Cp + 64,
                                           512 * Rp:512 * Rp + 512],
                                    lhsT=w3_t[64 * Rp:64 * Rp + 64,
                                              64 * t:64 * t + 64],
                                    rhs=x33[64 * Rp:64 * Rp + 64, Cp,
                                            QH * h + dy:QH * h + dy + QH,
                                            dx:dx + W],
                                    start=(t == 0), stop=(t == 8),
                                    tile_position=(64 * Rp, 64 * Cp),
                                    skip_group_check=True)
                    ptr = pt[:].rearrange("p (rp h w) -> p rp h w",
                                          rp=2, h=QH)
                    for rp in range(2):
                        nc.scalar.activation(
                            out=osb4[:, rp, QH * h:QH * h + QH, :],
                            in_=ptr[:, rp, :, :], func=AF.Prelu, alpha=0.1)
                nc.scalar.dma_start(out=y_d[:, 2048 * g:2048 * (g + 1)],
                                    in_=osb[:])

            # software-pipelined group loop; per step: evacs (conv2 of
            # g-1 on DVE, conv3 of g-2 on ACT) are emitted before this
            # group's conv1 and the NEXT group's gather adds, so the DVE
            # queue never blocks PSUM recycling behind gather DMAs
            Ast = {0: pool_gather(0)}
            for g in range(NG + 2):
                if 1 <= g < NG + 1:
                    conv2(g - 1)
                if g >= 2:
                    conv3(g - 2)
                if g < NG:
                    conv1(g, *Ast.pop(g))
                if g + 1 < NG:
                    Ast[g + 1] = pool_gather(g + 1)
    return nc


def _host_prep(feats, edges, w1, b1, w2, b2, w3, b3):
    import ml_dtypes

    feats = np.ascontiguousarray(np.asarray(feats, dtype=np.float32))
    edges = np.asarray(edges)
    w1 = np.asarray(w1, dtype=np.float32)
    w2 = np.asarray(w2, dtype=np.float32)
    w3 = np.asarray(w3, dtype=np.float32)

    contrib = [([], []) for _ in range(N)]
    for s, sg, d in edges.tolist():
        si = 0 if sg > 0 else 1
        contrib[d][si].append(s)
        contrib[s][si].append(d)
    md = np.array([max(len(contrib[n][0]), len(contrib[n][1]), 1)
                   for n in range(N)])

    # groups ascending by max-degree; slots within a group descending;
    # slot s = 2*(2Rp+j) + Cp
    slots = []
    for k in range(NCORES):
        nodes = sorted(range(NPC * k, NPC * (k + 1)), key=lambda n: md[n])
        gs = []
        for g in range(NG):
            gn = sorted(nodes[8 * g:8 * g + 8], key=lambda n: -md[n])
            gs.append(gn)
        slots.append(gs)

    geom = []
    for g in range(NG):
        rmax = max(max(md[slots[k][g][s]] for s in range(8))
                   for k in range(NCORES))
        rmax = max(int(rmax), 2)
        nas = []
        for r in range(rmax):
            if r < 2:
                nas.append(8)
            else:
                na = max(sum(1 for s in range(8) if md[slots[k][g][s]] > r)
                         for k in range(NCORES))
                nas.append(max(na, 1))
        geom.append(tuple(nas))
    geom = tuple(geom)

    featsN = feats.reshape(N, C, HW)
    tabN = np.concatenate([featsN, np.zeros((1, C, HW), np.float32)],
                          axis=0)
    tab_bf = tabN.astype(ml_dtypes.bfloat16)

    # weights (lhsT layouts, [64,64] pair tiles, replicated over row pairs)
    w1p = np.zeros((64, 9 * 64), np.float32)
    w1s = np.zeros((64, 2 * 9 * 64), np.float32)   # two parity variants
    w2l = np.zeros((64, 9 * 64), np.float32)
    w3l = np.zeros((64, 9 * 64), np.float32)
    for t in range(9):
        dy, dx = t // 3, t % 3
        for j in range(2):
            # pn: rows 32j+16sg+c -> cols 32j+m
            w1p[32 * j:32 * j + 32, 64 * t + 32 * j:64 * t + 32 * j + 32] = \
                w1[:, C:3 * C, dy, dx].T
            # w2: rows 32j+k -> cols 32j+m
            w2l[32 * j:32 * j + 32, 64 * t + 32 * j:64 * t + 32 * j + 32] = \
                w2[:, :, dy, dx].T
            # w3: rows 32j+k -> cols 32j+c (c<16)
            w3l[32 * j:32 * j + 32, 64 * t + 32 * j:64 * t + 32 * j + 16] = \
                w3[:, :, dy, dx].T
            # self variants: par p: rows 32p+16j+c -> cols 32j+m
            for p in range(2):
                w1s[32 * p + 16 * j:32 * p + 16 * j + 16,
                    576 * p + 64 * t + 32 * j:
                    576 * p + 64 * t + 32 * j + 32] = w1[:, 0:C, dy, dx].T
    rep = lambda a: np.ascontiguousarray(
        np.tile(a, (2, 1)).astype(ml_dtypes.bfloat16))

    in_maps = []
    for k in range(NCORES):
        m = {"w1p": rep(w1p), "w1s": rep(w1s), "w2l": rep(w2l),
             "w3l": rep(w3l)}
        # fown: pair P = g//2: p = 64Rp + 32*(g%2) + 16j + c,
        # free = 2048P + 1024Cp + px
        fo = np.zeros((128, (NG // 2) * 2048), ml_dtypes.bfloat16)
        for g in range(NG):
            for s in range(8):
                w_ = s // 2
                Rp, j, Cp = w_ // 2, w_ % 2, s % 2
                nsrc = slots[k][g][s]
                p0 = 64 * Rp + 32 * (g % 2) + 16 * j
                fo[p0:p0 + 16,
                   2048 * (g // 2) + 1024 * Cp:
                   2048 * (g // 2) + 1024 * Cp + 1024] = tab_bf[nsrc]
        m["fown"] = np.ascontiguousarray(fo)
        # gather slabs: row p = 32*(2Rp+j) + 16sg + c, col = 1024*Cp + px
        for g in range(NG):
            for r, na in enumerate(geom[g]):
                fw, odd = na // 2, na % 2
                nw = fw + odd
                srcs = np.full((2 * nw, 2), N, np.int64)  # [(w,sg), Cp]
                for s in range(8 if r < 2 else na):
                    w_, Cp = s // 2, s % 2
                    if w_ >= nw:
                        continue
                    nsrc = slots[k][g][s]
                    for sg in range(2):
                        lst = contrib[nsrc][sg]
                        if r < len(lst):
                            srcs[2 * w_ + sg, Cp] = lst[r]
                arr = tab_bf[srcs]  # [2nw, 2, C, HW]
                full = np.ascontiguousarray(
                    arr.transpose(0, 2, 1, 3).reshape(32 * nw, 2048))
                if fw > 0:
                    m[f"g{g}_{r}a"] = np.ascontiguousarray(
                        full[0:32 * fw, :])
                if odd:
                    m[f"g{g}_{r}b"] = np.ascontiguousarray(
                        full[32 * fw:32 * fw + 32, 0:1024])
        in_maps.append(m)
    return in_maps, slots, geom


def kernel(feats, edges, w1, b1, w2, b2, w3, b3):
    from concourse.bass_utils import run_bass_kernel_spmd

    with_bias = bool(np.any(np.asarray(b1)) or np.any(np.asarray(b2))
                     or np.any(np.asarray(b3)))
    assert not with_bias, "nonzero conv biases not implemented"

    in_maps, slots, geom = _host_prep(feats, edges, w1, b1, w2, b2, w3, b3)

    nc = _prog_cache.get(geom)
    if nc is None:
        nc = _build_program(geom)
        _prog_cache[geom] = nc

    import os
    trace = bool(os.environ.get("KERNEL_TRACE"))
    res = run_bass_kernel_spmd(nc, in_maps, core_ids=list(range(NCORES)),
                               trace=trace)
    if trace:
        global last_results
        last_results = res

    out = np.empty((N, C, H, W), np.float32)
    for k in range(NCORES):
        yk = np.asarray(res.results[k]["y"]).astype(np.float32)
        for g in range(NG):
            for s in range(8):
                w_ = s // 2
                Rp, j, Cp = w_ // 2, w_ % 2, s % 2
                n = slots[k][g][s]
                out[n] = yk[64 * Cp + 32 * j:64 * Cp + 32 * j + C,
                            2048 * g + 1024 * Rp:
                            2048 * g + 1024 * Rp + 1024].reshape(C, H, W)
    return out
